# revision 1
# baseline (speedup 1.0000x reference)
"""MeshFC kernel for 8x TRN2 NeuronCores.

Computes: out = inputs @ w + biases, where
  w[i,o] = ||in_pos[i]-out_pos[o]|| - ||init_in_pos[i]-init_out_pos[o]||

Sharding: tensor-parallel on the output dim (8 x 1024 columns). Each core:
  - generates its weight column block on-chip via the PE using the
    augmented-inner-product identity dist^2 = ||a||^2 - 2 a.b + ||b||^2
    (a single K=7 fp32 matmul per tile), sqrt on ScalarE, subtract on DVE
  - runs the main [4096,2048]x[2048,1024] matmul in float32r (FP22)
Host side: pre-transposes/pre-tiles inputs so every DMA is contiguous,
and concatenates the 8 per-core [4096,1024] outputs.
"""

import os
from contextlib import ExitStack

import numpy as np

NUM_IN, NUM_OUT, SD, BATCH = 2048, 8192, 5, 4096
N_CORES = 8
O_SHARD = NUM_OUT // N_CORES  # 1024
B_TILES = BATCH // 128  # 32
K_TILES = NUM_IN // 128  # 16
O_HALves = O_SHARD // 512  # 2

_CACHE = {}


def _build_bass(variant=""):
    import concourse.bass as bass  # noqa: F401
    import concourse.mybir as mybir
    from concourse import bacc
    from concourse.tile import TileContext

    fp32 = mybir.dt.float32
    fp32r = mybir.dt.float32r
    fp16 = mybir.dt.float16

    # main-matmul dtype: fp16 runs at 1 cycle/row (fp32r: 2, fp32: 4+) with
    # accuracy on par with fp22 (10-bit rounded vs 13-bit truncated mantissa)
    mmdt = fp32r if "fp32r" in variant else fp16

    # Bacc (not plain Bass): its compile() runs generate_event_semaphores +
    # move_matmul_waits_to_ldweights, which split multi-waits that exceed the
    # per-instruction HW sync-wait budget.
    nc = bacc.Bacc("TRN2", name="meshfc")

    xT = nc.dram_tensor("xT", [B_TILES, 128, NUM_IN], mmdt, kind="ExternalInput")
    # packed [aC | aI | bC | bI] along the free axis -> single DMA, single wait
    AB_W = 2 * NUM_IN + 2 * O_SHARD
    ab = nc.dram_tensor("ab", [7, AB_W], fp32, kind="ExternalInput")
    # [bias | ones(128)] packed on one partition
    bias = nc.dram_tensor("bias", [1, O_SHARD + 128], mmdt, kind="ExternalInput")
    out = nc.dram_tensor("out", [BATCH, O_SHARD], fp32, kind="ExternalOutput")

    with ExitStack() as ctx:
        tc = ctx.enter_context(TileContext(nc))
        const = ctx.enter_context(tc.tile_pool(name="const", bufs=1))
        wps = mps = tmp = xpool = opool = None
        if "nowgen" not in variant:
            wps = ctx.enter_context(tc.tile_pool(name="wps", bufs=2, space="PSUM"))
            tmp = ctx.enter_context(tc.tile_pool(name="tmp", bufs=2))
        if "nomm" not in variant:
            mps = ctx.enter_context(tc.tile_pool(name="mps", bufs=2, space="PSUM"))
            xpool = ctx.enter_context(tc.tile_pool(name="xp", bufs=3))
            opool = ctx.enter_context(tc.tile_pool(name="op", bufs=3))

        # --- constants ---
        ab_sb = const.tile([7, AB_W], fp32, name="ab_sb")
        nc.sync.dma_start(out=ab_sb, in_=ab[:, :])
        aC_sb = ab_sb[:, 0:NUM_IN]
        aI_sb = ab_sb[:, NUM_IN : 2 * NUM_IN]
        bC_sb = ab_sb[:, 2 * NUM_IN : 2 * NUM_IN + O_SHARD]
        bI_sb = ab_sb[:, 2 * NUM_IN + O_SHARD : AB_W]

        # bias lives on one partition; it is added into PSUM via a K=1 matmul
        biasones_sb = const.tile([1, O_SHARD + 128], mmdt, name="biasones_sb")
        nc.sync.dma_start(out=biasones_sb, in_=bias[:, :])
        bias_sb = biasones_sb[:, 0:O_SHARD]
        ones_sb = biasones_sb[:, O_SHARD : O_SHARD + 128]

        # resident weight block: [128, K_TILES, O_SHARD] = 8 MB
        # float32r so the DVE write rounds to FP22 for the fp32r matmul
        w_sb = const.tile([128, K_TILES, O_SHARD], mmdt, name="w_sb")

        # optional on-device repetition for slope timing (variant "repN")
        n_rep = 1
        for tok in variant.split(","):
            if tok.startswith("rep"):
                n_rep = int(tok[3:])

        # --- weight generation ---
        for _rep in range(n_rep):
            _build_body(nc, tc, variant, const, wps, mps, tmp, xpool, opool,
                        aC_sb, aI_sb, bC_sb, bI_sb, bias_sb, ones_sb, w_sb,
                        xT, out, fp32, mmdt)

    nc.finalize()
    return nc


def _build_body(nc, tc, variant, const, wps, mps, tmp, xpool, opool,
                aC_sb, aI_sb, bC_sb, bI_sb, bias_sb, ones_sb, w_sb,
                xT, out, fp32, mmdt):
    import concourse.mybir as mybir  # noqa: F401

    if True:
        if "nowgen" not in variant:
            for kt in range(K_TILES):
                for oh in range(O_HALves):
                    osl = slice(oh * 512, (oh + 1) * 512)
                    psC = wps.tile([128, 512], fp32, tag="psC", bufs=2)
                    psI = wps.tile([128, 512], fp32, tag="psI", bufs=2)
                    nc.tensor.matmul(
                        psC,
                        aC_sb[:, kt * 128 : (kt + 1) * 128],
                        bC_sb[:, osl],
                        start=True,
                        stop=True,
                    )
                    nc.tensor.matmul(
                        psI,
                        aI_sb[:, kt * 128 : (kt + 1) * 128],
                        bI_sb[:, osl],
                        start=True,
                        stop=True,
                    )
                    # clamp dist^2 to >=0 on DVE (HW fp32 rounding can push
                    # the closest pair slightly negative -> sqrt NaN), then
                    # sqrt in place in SBUF. In-place PSUM activation crashes
                    # the exec unit, so everything lands in SBUF tmps.
                    sC = tmp.tile([128, 512], fp32, tag="sC", bufs=2)
                    sI = tmp.tile([128, 512], fp32, tag="sI", bufs=2)
                    nc.vector.tensor_scalar_max(sC, psC, 0.0)
                    nc.vector.tensor_scalar_max(sI, psI, 0.0)
                    nc.scalar.sqrt(sC, sC)
                    nc.scalar.sqrt(sI, sI)
                    nc.vector.tensor_sub(w_sb[:, kt, osl], sC, sI)

        # --- main matmul: out[b,o] = sum_k x[b,k] w[k,o] (+bias) ---
        if "nomm" in variant:
            return
        for bt in range(B_TILES):
            xt = xpool.tile([128, NUM_IN], mmdt, name="xt")
            if "nodma" not in variant:
                nc.sync.dma_start(out=xt, in_=xT[bt])
            ot = opool.tile([128, O_SHARD], fp32, name="ot")
            # pre-touch: absorbs the out-DMA slot-release wait on ScalarE so
            # the real drains below stay within the HW sync-wait slot limit
            if "nodrain" not in variant:
                nc.scalar.mul(ot[0:1, 0:1], ot[0:1, 0:1], 0.0)
            for oh in range(O_HALves):
                osl = slice(oh * 512, (oh + 1) * 512)
                ps = mps.tile([128, 512], fp32, tag="ps", bufs=2)
                for kt in range(K_TILES):
                    nc.tensor.matmul(
                        ps,
                        xt[:, kt * 128 : (kt + 1) * 128],
                        w_sb[:, kt, osl],
                        start=(kt == 0),
                        stop=("nobias" in variant and kt == K_TILES - 1),
                    )
                # += bias (broadcast over rows via rank-1 matmul)
                if "nobias" not in variant:
                    nc.tensor.matmul(
                        ps, ones_sb[:, :], bias_sb[:, osl], start=False, stop=True
                    )
                if "nodrain" not in variant:
                    nc.scalar.copy(ot[:, osl], ps)
            if "nodrain" not in variant:
                nc.sync.dma_start(out=out[bt * 128 : (bt + 1) * 128, :], in_=ot)


def _prep_inputs(inputs, init_in_pos, init_out_pos, in_pos, out_pos, biases,
                 mm_np_dt=np.float16):
    x = np.ascontiguousarray(np.asarray(inputs, dtype=np.float32))
    a = np.asarray(in_pos, dtype=np.float32).reshape(NUM_IN, SD)
    a0 = np.asarray(init_in_pos, dtype=np.float32).reshape(NUM_IN, SD)
    b = np.asarray(out_pos, dtype=np.float32).reshape(NUM_OUT, SD)
    b0 = np.asarray(init_out_pos, dtype=np.float32).reshape(NUM_OUT, SD)
    bias = np.asarray(biases, dtype=np.float32).reshape(NUM_OUT)

    # [bt, p, kt*128+b'] = x[bt*128+b', kt*128+p]
    xT = np.ascontiguousarray(
        x.reshape(B_TILES, 128, K_TILES, 128).transpose(0, 3, 2, 1).astype(mm_np_dt)
    ).reshape(B_TILES, 128, NUM_IN)

    def aug_a(p):
        return np.concatenate(
            [p.T, (p * p).sum(1)[None, :], np.ones((1, p.shape[0]), np.float32)], 0
        ).astype(np.float32)

    def aug_b(q):
        return np.concatenate(
            [-2.0 * q.T, np.ones((1, q.shape[0]), np.float32), (q * q).sum(1)[None, :]],
            0,
        ).astype(np.float32)

    aCv, aIv = aug_a(a), aug_a(a0)
    bC_full, bI_full = aug_b(b), aug_b(b0)

    in_maps = []
    for c in range(N_CORES):
        sl = slice(c * O_SHARD, (c + 1) * O_SHARD)
        ab = np.ascontiguousarray(
            np.concatenate([aCv, aIv, bC_full[:, sl], bI_full[:, sl]], axis=1)
        )
        in_maps.append(
            {
                "xT": xT,
                "ab": ab,
                "bias": np.ascontiguousarray(
                    np.concatenate([bias[sl], np.ones(128, np.float32)]).astype(
                        mm_np_dt
                    )
                )[None, :],
            }
        )
    return in_maps


def _run(in_maps, trace=False):
    from concourse.bass_utils import run_bass_kernel_spmd

    if "nc" not in _CACHE:
        _CACHE["nc"] = _build_bass()
    nc = _CACHE["nc"]
    res = run_bass_kernel_spmd(
        nc, in_maps, core_ids=list(range(N_CORES)), trace=trace
    )
    outs = [r["out"] for r in res.results]
    return np.concatenate(outs, axis=1), res


def kernel(**inputs) -> np.ndarray:
    in_maps = _prep_inputs(**inputs)
    out, _ = _run(in_maps, trace=bool(os.environ.get("MESHFC_TRACE")))
    return out



# revision 9
# speedup vs baseline: 1.2140x; 1.2140x over previous
"""MeshFC kernel for 8x TRN2 NeuronCores.

Computes: out = inputs @ w + biases, where
  w[i,o] = ||in_pos[i]-out_pos[o]|| - ||init_in_pos[i]-init_out_pos[o]||

Sharding: tensor-parallel on the output dim (8 x 1024 columns).

Per-core pipeline (all PE matmuls at 1 cycle/row):
  - weight gen via the difference form
        t = ||a0-b0||^2 + EPS   (3 fp32r matmuls: hi/lo mantissa split of
                                 both operands reconstructs ~fp32 accuracy;
                                 a single fp32r matmul truncates operands
                                 to 13-bit mantissa -> |err| ~1e-3, which
                                 swamps near-coincident pairs)
        D = ||a-b||^2 - ||a0-b0||^2   (1 fp32r matmul, cancellation-free
                                 aug vectors: a, a-a0 paired with -2(b-b0),
                                 -2b0 -> every product is O(delta)=O(0.01))
        w = sqrt(t+D) - sqrt(t)
    EPS=3e-4 keeps both sqrt args positive against the ~5e-5 residual
    matmul error (no DVE clamps, no NaN); it distorts w by ~EPS*w/(2d^2),
    absmax ~1e-3 on w for the closest pairs - negligible in the output.
    The t-error cancels between the two sqrts since u adds D on top of
    the same computed t.
  - main [4096,2048]x[2048,1024] matmul in fp16, oh-phased (512-col
    halves) so weight-gen DVE/Act work overlaps the matmul stream; x is
    resident in SBUF (128 KiB/partition) so phase 2 issues no input DMA.
  - biases are added on the host (they are zeros in this problem).
"""

import os
from contextlib import ExitStack

import numpy as np

NUM_IN, NUM_OUT, SD, BATCH = 2048, 8192, 5, 4096
N_CORES = 8
O_SHARD = NUM_OUT // N_CORES  # 1024
B_TILES = BATCH // 128  # 32
K_TILES = NUM_IN // 128  # 16
O_HALVES = O_SHARD // 512  # 2
EPS = 3e-4

# packed ab columns: [aD | aTh | aTl (2048 each) | bD | bTh | bTl (1024 each)]
AB_W = 3 * NUM_IN + 3 * O_SHARD

_CACHE = {}


def _split13(v):
    """hi/lo split at 13 explicit mantissa bits (fp32r/FP22 exact)."""
    v = np.ascontiguousarray(v.astype(np.float32))
    hi = (v.view(np.uint32) & np.uint32(0xFFFFE000)).view(np.float32)
    return hi, (v - hi).astype(np.float32)


def _build_bass(variant=""):
    import concourse.bass as bass  # noqa: F401
    import concourse.mybir as mybir
    from concourse import bacc
    from concourse.tile import TileContext

    fp32 = mybir.dt.float32
    fp32r = mybir.dt.float32r
    fp16 = mybir.dt.float16
    af = mybir.ActivationFunctionType

    nc = bacc.Bacc("TRN2", name="meshfc")

    xT = nc.dram_tensor("xT", [B_TILES, 128, NUM_IN], fp16, kind="ExternalInput")
    ab = nc.dram_tensor("ab", [12, AB_W], fp32r, kind="ExternalInput")
    out = nc.dram_tensor("out", [BATCH, O_SHARD], fp32, kind="ExternalOutput")

    n_rep = 1
    for tok in variant.split(","):
        if tok.startswith("rep"):
            n_rep = int(tok[3:])

    with ExitStack() as ctx:
        tc = ctx.enter_context(TileContext(nc))
        const = ctx.enter_context(tc.tile_pool(name="const", bufs=1))
        wps = tmp = mps = opool = None
        if "nowgen" not in variant:
            wps = ctx.enter_context(tc.tile_pool(name="wps", bufs=2, space="PSUM"))
            tmp = ctx.enter_context(tc.tile_pool(name="tmp", bufs=2))
        if "nomm" not in variant:
            mps = ctx.enter_context(tc.tile_pool(name="mps", bufs=3, space="PSUM"))
            opool = ctx.enter_context(tc.tile_pool(name="op", bufs=3))

        # --- constants ---
        ab_sb = const.tile([12, AB_W], fp32r, name="ab_sb")
        nc.sync.dma_start(out=ab_sb, in_=ab[:, :])
        c = [0, NUM_IN, 2 * NUM_IN, 3 * NUM_IN, 3 * NUM_IN + O_SHARD,
             3 * NUM_IN + 2 * O_SHARD, AB_W]
        aD_sb = ab_sb[:, c[0] : c[1]]
        aTh_sb = ab_sb[0:7, c[1] : c[2]]
        aTl_sb = ab_sb[0:7, c[2] : c[3]]
        bD_sb = ab_sb[:, c[3] : c[4]]
        bTh_sb = ab_sb[0:7, c[4] : c[5]]
        bTl_sb = ab_sb[0:7, c[5] : c[6]]

        # x streamed through an 8-deep pool (32 KiB/part): each oh-phase
        # re-DMAs the 32 x tiles; prefetch stays ~8 tiles ahead of the PE
        xpool = None
        if "nomm" not in variant:
            xpool = ctx.enter_context(tc.tile_pool(name="xp", bufs=8))

        # per-half weight blocks (separate tiles so tile-level deps give
        # oh-granular pipelining even if subtile tracking is conservative)
        whs = [
            const.tile([128, K_TILES, 512], fp16, name=f"w{oh}")
            for oh in range(O_HALVES)
        ]

        for _rep in range(n_rep):
            _build_body(nc, variant, af, wps, tmp, mps, opool, xpool,
                        aD_sb, aTh_sb, aTl_sb, bD_sb, bTh_sb, bTl_sb,
                        whs, xT, out, fp32, fp16)

    nc.finalize()
    return nc


def _build_body(nc, variant, af, wps, tmp, mps, opool, xpool,
                aD_sb, aTh_sb, aTl_sb, bD_sb, bTh_sb, bTl_sb,
                whs, xT, out, fp32, fp16):
    # --- weight generation: w = sqrt(t+D) - sqrt(t) ---
    if "nowgen" not in variant:
        for oh in range(O_HALVES):
            osl = slice(oh * 512, (oh + 1) * 512)
            for kt in range(K_TILES):
                ksl = slice(kt * 128, (kt + 1) * 128)
                psT = wps.tile([128, 512], fp32, tag="psT", bufs=2)
                psD = wps.tile([128, 512], fp32, tag="psD", bufs=2)
                nc.tensor.matmul(psT, aTh_sb[:, ksl], bTh_sb[:, osl],
                                 start=True, stop=False)
                nc.tensor.matmul(psT, aTh_sb[:, ksl], bTl_sb[:, osl],
                                 start=False, stop=False)
                nc.tensor.matmul(psT, aTl_sb[:, ksl], bTh_sb[:, osl],
                                 start=False, stop=True)
                nc.tensor.matmul(psD, aD_sb[:, ksl], bD_sb[:, osl],
                                 start=True, stop=True)
                u = tmp.tile([128, 512], fp32, tag="u", bufs=2)
                sI = tmp.tile([128, 512], fp32, tag="sI", bufs=2)
                # alternate the t-copy between DVE and Act: per-tile chain is
                # then ~2.5 ops on each engine, matching the PE's 852ns/tile
                if kt % 2 == 0:
                    nc.vector.tensor_copy(sI, psT)
                else:
                    nc.scalar.activation(sI, psT, af.Copy)
                nc.vector.tensor_add(u, sI, psD)  # SBUF + PSUM (one-PSUM ok)
                nc.scalar.sqrt(sI, sI)
                nc.scalar.sqrt(u, u)
                nc.vector.tensor_sub(whs[oh][:, kt, :], u, sI)

    # --- main matmul, oh-phased: out[b, osl] = x[b, :] @ w[:, osl] ---
    if "nomm" in variant:
        return
    for oh in range(O_HALVES):
        osl = slice(oh * 512, (oh + 1) * 512)
        for bt in range(B_TILES):
            xt = xpool.tile([128, NUM_IN], fp16, tag="xt", bufs=8)
            nc.sync.dma_start(out=xt, in_=xT[bt])
            ps = mps.tile([128, 512], fp32, tag="ps", bufs=3)
            for kt in range(K_TILES):
                nc.tensor.matmul(
                    ps,
                    xt[:, kt * 128 : (kt + 1) * 128],
                    whs[oh][:, kt, :],
                    start=(kt == 0),
                    stop=(kt == K_TILES - 1),
                )
            ot = opool.tile([128, 512], fp32, tag="ot", bufs=3)
            nc.scalar.copy(ot, ps)
            nc.sync.dma_start(out=out[bt * 128 : (bt + 1) * 128, osl], in_=ot)


def _prep_inputs(inputs, init_in_pos, init_out_pos, in_pos, out_pos, biases):
    x = np.asarray(inputs, dtype=np.float32)
    a = np.asarray(in_pos, dtype=np.float64).reshape(NUM_IN, SD)
    a0 = np.asarray(init_in_pos, dtype=np.float64).reshape(NUM_IN, SD)
    b = np.asarray(out_pos, dtype=np.float64).reshape(NUM_OUT, SD)
    b0 = np.asarray(init_out_pos, dtype=np.float64).reshape(NUM_OUT, SD)
    bias = np.asarray(biases, dtype=np.float32).reshape(NUM_OUT)

    # xT[bt, d, kt*128+b'] = x[bt*128+b', kt*128+d]
    xT = np.ascontiguousarray(
        x.reshape(B_TILES, 128, K_TILES, 128).transpose(0, 3, 2, 1)
        .astype(np.float16)
    ).reshape(B_TILES, 128, NUM_IN)

    da, db = a - a0, b - b0
    Sa = (a * a).sum(1) - (a0 * a0).sum(1)
    Sb = (b * b).sum(1) - (b0 * b0).sum(1)

    ones_i = np.ones(NUM_IN)
    ones_o = np.ones(NUM_OUT)
    # D[i,o] = a.(-2db) + da.(-2b0) + Sa*1 + 1*Sb  = dist^2 - dist0^2
    aD = np.concatenate([a.T, da.T, Sa[None, :], ones_i[None, :]], 0)
    bD_full = np.concatenate([-2.0 * db.T, -2.0 * b0.T, ones_o[None, :],
                              Sb[None, :]], 0)
    # t[i,o] = a0.(-2b0) + |a0|^2*1 + 1*(|b0|^2+EPS) = dist0^2 + EPS,
    # via hi/lo split fp32r matmuls: t = ah.bh + ah.bl + al.bh
    aT = np.concatenate([a0.T, (a0 * a0).sum(1)[None, :], ones_i[None, :]], 0)
    bT_full = np.concatenate([-2.0 * b0.T, ones_o[None, :],
                              ((b0 * b0).sum(1) + EPS)[None, :]], 0)
    aTh, aTl = _split13(aT)
    bTh_full, bTl_full = _split13(bT_full)
    # pad 7-row pieces to 12 rows so everything packs into one [12,*] DMA
    pad_i = np.zeros((5, NUM_IN), np.float32)
    pad_o = np.zeros((5, NUM_OUT), np.float32)
    aTh12 = np.concatenate([aTh, pad_i], 0)
    aTl12 = np.concatenate([aTl, pad_i], 0)
    bTh12_full = np.concatenate([bTh_full, pad_o], 0)
    bTl12_full = np.concatenate([bTl_full, pad_o], 0)

    in_maps = []
    for c in range(N_CORES):
        sl = slice(c * O_SHARD, (c + 1) * O_SHARD)
        ab = np.ascontiguousarray(
            np.concatenate(
                [aD.astype(np.float32), aTh12, aTl12,
                 bD_full[:, sl].astype(np.float32),
                 bTh12_full[:, sl], bTl12_full[:, sl]], axis=1
            ).astype(np.float32)
        )
        in_maps.append({"xT": xT, "ab": ab})
    return in_maps, bias


def _run(in_maps, trace=False):
    from concourse.bass_utils import run_bass_kernel_spmd

    variant = os.environ.get("MESHFC_VARIANT", "")
    key = ("nc", variant)
    if key not in _CACHE:
        _CACHE[key] = _build_bass(variant)
    nc = _CACHE[key]
    res = run_bass_kernel_spmd(
        nc, in_maps, core_ids=list(range(N_CORES)), trace=trace
    )
    outs = [r["out"] for r in res.results]
    return np.concatenate(outs, axis=1), res


def kernel(**inputs) -> np.ndarray:
    in_maps, bias = _prep_inputs(**inputs)
    out, _ = _run(in_maps, trace=bool(os.environ.get("MESHFC_TRACE")))
    if bias.any():
        out = out + bias[None, :]
    return out


# revision 10
# speedup vs baseline: 1.2589x; 1.0370x over previous
"""MeshFC kernel for 8x TRN2 NeuronCores.

Computes: out = inputs @ w + biases, where
  w[i,o] = ||in_pos[i]-out_pos[o]|| - ||init_in_pos[i]-init_out_pos[o]||

Sharding: tensor-parallel on the output dim (8 x 1024 columns).

Per-core pipeline (all PE matmuls at 1 cycle/row):
  - weight gen via the difference form
        t = ||a0-b0||^2         (3 fp32r matmuls: hi/lo mantissa split of
                                 both operands reconstructs ~fp32 accuracy;
                                 a single fp32r matmul truncates operands
                                 to 13-bit mantissa -> |err| ~1e-3, which
                                 swamps near-coincident pairs)
        D = ||a-b||^2 - ||a0-b0||^2   (1 fp32r matmul, cancellation-free
                                 aug vectors: a, a-a0 paired with -2(b-b0),
                                 -2b0 -> every product is O(delta)=O(0.01))
        w = sqrt(t+D) - sqrt(t)
    Both sqrt args are clamped to >=0 (the ~2e-5 residual t error can go
    negative on near-coincident pairs -> NaN otherwise; an eps shift is
    NOT usable: any eps creates sqrt(eps)-scale w errors on pairs with
    dist0^2 < eps). The t-error otherwise cancels between the two sqrts
    since u adds D on top of the same computed t.
  - main [4096,2048]x[2048,1024] matmul in fp16, oh-phased (512-col
    halves) so weight-gen DVE/Act work overlaps the matmul stream; x is
    resident in SBUF (128 KiB/partition) so phase 2 issues no input DMA.
  - biases are added on the host (they are zeros in this problem).
"""

import os
from contextlib import ExitStack

import numpy as np

NUM_IN, NUM_OUT, SD, BATCH = 2048, 8192, 5, 4096
N_CORES = 8
O_SHARD = NUM_OUT // N_CORES  # 1024
B_TILES = BATCH // 128  # 32
K_TILES = NUM_IN // 128  # 16
O_HALVES = O_SHARD // 512  # 2
EPS = 0.0  # clamps handle negativity; any eps>0 costs sqrt(eps) errors

# packed ab columns: [aD | aTh | aTl (2048 each) | bD | bTh | bTl (1024 each)]
AB_W = 3 * NUM_IN + 3 * O_SHARD

_CACHE = {}


def _split13(v):
    """hi/lo split at 13 explicit mantissa bits (fp32r/FP22 exact)."""
    v = np.ascontiguousarray(v.astype(np.float32))
    hi = (v.view(np.uint32) & np.uint32(0xFFFFE000)).view(np.float32)
    return hi, (v - hi).astype(np.float32)


def _build_bass(variant=""):
    import concourse.bass as bass  # noqa: F401
    import concourse.mybir as mybir
    from concourse import bacc
    from concourse.tile import TileContext

    fp32 = mybir.dt.float32
    fp32r = mybir.dt.float32r
    fp16 = mybir.dt.float16
    af = mybir.ActivationFunctionType

    nc = bacc.Bacc("TRN2", name="meshfc")

    xT = nc.dram_tensor("xT", [B_TILES, 128, NUM_IN], fp16, kind="ExternalInput")
    ab = nc.dram_tensor("ab", [12, AB_W], fp32r, kind="ExternalInput")
    out = nc.dram_tensor("out", [BATCH, O_SHARD], fp32, kind="ExternalOutput")

    n_rep = 1
    for tok in variant.split(","):
        if tok.startswith("rep"):
            n_rep = int(tok[3:])

    with ExitStack() as ctx:
        tc = ctx.enter_context(TileContext(nc))
        const = ctx.enter_context(tc.tile_pool(name="const", bufs=1))
        wps = tmp = mps = opool = None
        if "nowgen" not in variant:
            wps = ctx.enter_context(tc.tile_pool(name="wps", bufs=2, space="PSUM"))
            tmp = ctx.enter_context(tc.tile_pool(name="tmp", bufs=2))
        if "nomm" not in variant:
            mps = ctx.enter_context(tc.tile_pool(name="mps", bufs=3, space="PSUM"))
            opool = ctx.enter_context(tc.tile_pool(name="op", bufs=3))

        # --- constants ---
        ab_sb = const.tile([12, AB_W], fp32r, name="ab_sb")
        nc.sync.dma_start(out=ab_sb, in_=ab[:, :])
        c = [0, NUM_IN, 2 * NUM_IN, 3 * NUM_IN, 3 * NUM_IN + O_SHARD,
             3 * NUM_IN + 2 * O_SHARD, AB_W]
        aD_sb = ab_sb[:, c[0] : c[1]]
        aTh_sb = ab_sb[0:7, c[1] : c[2]]
        aTl_sb = ab_sb[0:7, c[2] : c[3]]
        bD_sb = ab_sb[:, c[3] : c[4]]
        bTh_sb = ab_sb[0:7, c[4] : c[5]]
        bTl_sb = ab_sb[0:7, c[5] : c[6]]

        # x streamed through an 8-deep pool (32 KiB/part): each oh-phase
        # re-DMAs the 32 x tiles; prefetch stays ~8 tiles ahead of the PE
        xpool = None
        if "nomm" not in variant:
            xpool = ctx.enter_context(tc.tile_pool(name="xp", bufs=8))

        # per-half weight blocks (separate tiles so tile-level deps give
        # oh-granular pipelining even if subtile tracking is conservative)
        whs = [
            const.tile([128, K_TILES, 512], fp16, name=f"w{oh}")
            for oh in range(O_HALVES)
        ]

        for _rep in range(n_rep):
            _build_body(nc, variant, af, wps, tmp, mps, opool, xpool,
                        aD_sb, aTh_sb, aTl_sb, bD_sb, bTh_sb, bTl_sb,
                        whs, xT, out, fp32, fp16)

    nc.finalize()
    return nc


def _build_body(nc, variant, af, wps, tmp, mps, opool, xpool,
                aD_sb, aTh_sb, aTl_sb, bD_sb, bTh_sb, bTl_sb,
                whs, xT, out, fp32, fp16):
    # --- weight generation: w = sqrt(t+D) - sqrt(t) ---
    if "nowgen" not in variant:
        for oh in range(O_HALVES):
            osl = slice(oh * 512, (oh + 1) * 512)
            for kt in range(K_TILES):
                ksl = slice(kt * 128, (kt + 1) * 128)
                psT = wps.tile([128, 512], fp32, tag="psT", bufs=2)
                psD = wps.tile([128, 512], fp32, tag="psD", bufs=2)
                nc.tensor.matmul(psT, aTh_sb[:, ksl], bTh_sb[:, osl],
                                 start=True, stop=False)
                nc.tensor.matmul(psT, aTh_sb[:, ksl], bTl_sb[:, osl],
                                 start=False, stop=False)
                nc.tensor.matmul(psT, aTl_sb[:, ksl], bTh_sb[:, osl],
                                 start=False, stop=True)
                nc.tensor.matmul(psD, aD_sb[:, ksl], bD_sb[:, osl],
                                 start=True, stop=True)
                u = tmp.tile([128, 512], fp32, tag="u", bufs=2)
                sI = tmp.tile([128, 512], fp32, tag="sI", bufs=2)
                # t-clamp doubles as the PSUM->SBUF move; u is clamped
                # in-place on ScalarE so each engine carries 3 ops/tile
                nc.vector.tensor_scalar_max(sI, psT, 0.0)
                nc.vector.tensor_add(u, sI, psD)  # SBUF + PSUM (one-PSUM ok)
                nc.scalar.activation(u, u, af.Relu)
                nc.scalar.sqrt(sI, sI)
                nc.scalar.sqrt(u, u)
                nc.vector.tensor_sub(whs[oh][:, kt, :], u, sI)

    # --- main matmul, oh-phased: out[b, osl] = x[b, :] @ w[:, osl] ---
    if "nomm" in variant:
        return
    for oh in range(O_HALVES):
        osl = slice(oh * 512, (oh + 1) * 512)
        for bt in range(B_TILES):
            xt = xpool.tile([128, NUM_IN], fp16, tag="xt", bufs=8)
            nc.sync.dma_start(out=xt, in_=xT[bt])
            ps = mps.tile([128, 512], fp32, tag="ps", bufs=3)
            for kt in range(K_TILES):
                nc.tensor.matmul(
                    ps,
                    xt[:, kt * 128 : (kt + 1) * 128],
                    whs[oh][:, kt, :],
                    start=(kt == 0),
                    stop=(kt == K_TILES - 1),
                )
            ot = opool.tile([128, 512], fp32, tag="ot", bufs=3)
            nc.scalar.copy(ot, ps)
            nc.sync.dma_start(out=out[bt * 128 : (bt + 1) * 128, osl], in_=ot)


def _prep_inputs(inputs, init_in_pos, init_out_pos, in_pos, out_pos, biases):
    x = np.asarray(inputs, dtype=np.float32)
    a = np.asarray(in_pos, dtype=np.float64).reshape(NUM_IN, SD)
    a0 = np.asarray(init_in_pos, dtype=np.float64).reshape(NUM_IN, SD)
    b = np.asarray(out_pos, dtype=np.float64).reshape(NUM_OUT, SD)
    b0 = np.asarray(init_out_pos, dtype=np.float64).reshape(NUM_OUT, SD)
    bias = np.asarray(biases, dtype=np.float32).reshape(NUM_OUT)

    # xT[bt, d, kt*128+b'] = x[bt*128+b', kt*128+d]
    xT = np.ascontiguousarray(
        x.reshape(B_TILES, 128, K_TILES, 128).transpose(0, 3, 2, 1)
        .astype(np.float16)
    ).reshape(B_TILES, 128, NUM_IN)

    da, db = a - a0, b - b0
    Sa = (a * a).sum(1) - (a0 * a0).sum(1)
    Sb = (b * b).sum(1) - (b0 * b0).sum(1)

    ones_i = np.ones(NUM_IN)
    ones_o = np.ones(NUM_OUT)
    # D[i,o] = a.(-2db) + da.(-2b0) + Sa*1 + 1*Sb  = dist^2 - dist0^2
    aD = np.concatenate([a.T, da.T, Sa[None, :], ones_i[None, :]], 0)
    bD_full = np.concatenate([-2.0 * db.T, -2.0 * b0.T, ones_o[None, :],
                              Sb[None, :]], 0)
    # t[i,o] = a0.(-2b0) + |a0|^2*1 + 1*(|b0|^2+EPS) = dist0^2 + EPS,
    # via hi/lo split fp32r matmuls: t = ah.bh + ah.bl + al.bh
    aT = np.concatenate([a0.T, (a0 * a0).sum(1)[None, :], ones_i[None, :]], 0)
    bT_full = np.concatenate([-2.0 * b0.T, ones_o[None, :],
                              ((b0 * b0).sum(1) + EPS)[None, :]], 0)
    aTh, aTl = _split13(aT)
    bTh_full, bTl_full = _split13(bT_full)
    # pad 7-row pieces to 12 rows so everything packs into one [12,*] DMA
    pad_i = np.zeros((5, NUM_IN), np.float32)
    pad_o = np.zeros((5, NUM_OUT), np.float32)
    aTh12 = np.concatenate([aTh, pad_i], 0)
    aTl12 = np.concatenate([aTl, pad_i], 0)
    bTh12_full = np.concatenate([bTh_full, pad_o], 0)
    bTl12_full = np.concatenate([bTl_full, pad_o], 0)

    in_maps = []
    for c in range(N_CORES):
        sl = slice(c * O_SHARD, (c + 1) * O_SHARD)
        ab = np.ascontiguousarray(
            np.concatenate(
                [aD.astype(np.float32), aTh12, aTl12,
                 bD_full[:, sl].astype(np.float32),
                 bTh12_full[:, sl], bTl12_full[:, sl]], axis=1
            ).astype(np.float32)
        )
        in_maps.append({"xT": xT, "ab": ab})
    return in_maps, bias


def _run(in_maps, trace=False):
    from concourse.bass_utils import run_bass_kernel_spmd

    variant = os.environ.get("MESHFC_VARIANT", "")
    key = ("nc", variant)
    if key not in _CACHE:
        _CACHE[key] = _build_bass(variant)
    nc = _CACHE[key]
    res = run_bass_kernel_spmd(
        nc, in_maps, core_ids=list(range(N_CORES)), trace=trace
    )
    outs = [r["out"] for r in res.results]
    return np.concatenate(outs, axis=1), res


def kernel(**inputs) -> np.ndarray:
    in_maps, bias = _prep_inputs(**inputs)
    out, _ = _run(in_maps, trace=bool(os.environ.get("MESHFC_TRACE")))
    if bias.any():
        out = out + bias[None, :]
    return out


# revision 11
# speedup vs baseline: 1.2781x; 1.0153x over previous
"""MeshFC kernel for 8x TRN2 NeuronCores.

Computes: out = inputs @ w + biases, where
  w[i,o] = ||in_pos[i]-out_pos[o]|| - ||init_in_pos[i]-init_out_pos[o]||

Sharding: tensor-parallel on the output dim (8 x 1024 columns).

Per-core pipeline (all PE matmuls at 1 cycle/row):
  - weight gen via the difference form
        t = ||a0-b0||^2         (3 fp16 matmuls: hi/lo mantissa split of
                                 both operands gives ~22-bit accuracy, err
                                 ~1e-5; fp16/fp32r single matmuls err at
                                 ~1e-3/2e-3, swamping near-coincident
                                 pairs; fp16 streams 1 cyc/row vs fp32r's
                                 measured 2 cyc/row)
        D = ||a-b||^2 - ||a0-b0||^2   (1 fp16 matmul, cancellation-free
                                 aug vectors: a, a-a0 paired with -2(b-b0),
                                 -2b0 -> every product is O(delta)=O(0.01),
                                 so fp16 rounding costs only ~1e-5 on w)
        w = sqrt(t+D) - sqrt(t)
    Both sqrt args are clamped to >=0 (the ~2e-5 residual t error can go
    negative on near-coincident pairs -> NaN otherwise; an eps shift is
    NOT usable: any eps creates sqrt(eps)-scale w errors on pairs with
    dist0^2 < eps). The t-error otherwise cancels between the two sqrts
    since u adds D on top of the same computed t.
  - main [4096,2048]x[2048,1024] matmul in fp16, oh-phased (512-col
    halves) so weight-gen DVE/Act work overlaps the matmul stream; x is
    resident in SBUF (128 KiB/partition) so phase 2 issues no input DMA.
  - biases are added on the host (they are zeros in this problem).
"""

import os
from contextlib import ExitStack

import numpy as np

NUM_IN, NUM_OUT, SD, BATCH = 2048, 8192, 5, 4096
N_CORES = 8
O_SHARD = NUM_OUT // N_CORES  # 1024
B_TILES = BATCH // 128  # 32
K_TILES = NUM_IN // 128  # 16
O_HALVES = O_SHARD // 512  # 2
EPS = 0.0  # clamps handle negativity; any eps>0 costs sqrt(eps) errors

# packed ab columns: [aD | aTh | aTl (2048 each) | bD | bTh | bTl (1024 each)]
AB_W = 3 * NUM_IN + 3 * O_SHARD

_CACHE = {}


def _split16(v):
    """hi/lo split at fp16 precision: v ~= hi + lo with both fp16."""
    hi = v.astype(np.float16)
    lo = (v - hi.astype(np.float64)).astype(np.float16)
    return hi, lo


def _build_bass(variant=""):
    import concourse.bass as bass  # noqa: F401
    import concourse.mybir as mybir
    from concourse import bacc
    from concourse.tile import TileContext

    fp32 = mybir.dt.float32
    fp32r = mybir.dt.float32r
    fp16 = mybir.dt.float16
    af = mybir.ActivationFunctionType

    nc = bacc.Bacc("TRN2", name="meshfc")

    xT = nc.dram_tensor("xT", [B_TILES, 128, NUM_IN], fp16, kind="ExternalInput")
    ab = nc.dram_tensor("ab", [12, AB_W], fp16, kind="ExternalInput")
    out = nc.dram_tensor("out", [BATCH, O_SHARD], fp32, kind="ExternalOutput")

    n_rep = 1
    for tok in variant.split(","):
        if tok.startswith("rep"):
            n_rep = int(tok[3:])

    with ExitStack() as ctx:
        tc = ctx.enter_context(TileContext(nc))
        const = ctx.enter_context(tc.tile_pool(name="const", bufs=1))
        wps = tmp = mps = opool = None
        if "nowgen" not in variant:
            wps = ctx.enter_context(tc.tile_pool(name="wps", bufs=2, space="PSUM"))
            tmp = ctx.enter_context(tc.tile_pool(name="tmp", bufs=2))
        if "nomm" not in variant:
            mps = ctx.enter_context(tc.tile_pool(name="mps", bufs=3, space="PSUM"))
            opool = ctx.enter_context(tc.tile_pool(name="op", bufs=3))

        # --- constants ---
        ab_sb = const.tile([12, AB_W], fp16, name="ab_sb")
        nc.sync.dma_start(out=ab_sb, in_=ab[:, :])
        c = [0, NUM_IN, 2 * NUM_IN, 3 * NUM_IN, 3 * NUM_IN + O_SHARD,
             3 * NUM_IN + 2 * O_SHARD, AB_W]
        aD_sb = ab_sb[:, c[0] : c[1]]
        aTh_sb = ab_sb[0:7, c[1] : c[2]]
        aTl_sb = ab_sb[0:7, c[2] : c[3]]
        bD_sb = ab_sb[:, c[3] : c[4]]
        bTh_sb = ab_sb[0:7, c[4] : c[5]]
        bTl_sb = ab_sb[0:7, c[5] : c[6]]

        # x streamed through an 8-deep pool (32 KiB/part): each oh-phase
        # re-DMAs the 32 x tiles; prefetch stays ~8 tiles ahead of the PE
        xpool = None
        if "nomm" not in variant:
            xpool = ctx.enter_context(tc.tile_pool(name="xp", bufs=8))

        # per-half weight blocks (separate tiles so tile-level deps give
        # oh-granular pipelining even if subtile tracking is conservative)
        whs = [
            const.tile([128, K_TILES, 512], fp16, name=f"w{oh}")
            for oh in range(O_HALVES)
        ]

        for _rep in range(n_rep):
            _build_body(nc, variant, af, wps, tmp, mps, opool, xpool,
                        aD_sb, aTh_sb, aTl_sb, bD_sb, bTh_sb, bTl_sb,
                        whs, xT, out, fp32, fp16)

    nc.finalize()
    return nc


def _build_body(nc, variant, af, wps, tmp, mps, opool, xpool,
                aD_sb, aTh_sb, aTl_sb, bD_sb, bTh_sb, bTl_sb,
                whs, xT, out, fp32, fp16):
    # --- weight generation: w = sqrt(t+D) - sqrt(t) ---
    if "nowgen" not in variant:
        for oh in range(O_HALVES):
            osl = slice(oh * 512, (oh + 1) * 512)
            for kt in range(K_TILES):
                ksl = slice(kt * 128, (kt + 1) * 128)
                psT = wps.tile([128, 512], fp32, tag="psT", bufs=2)
                psD = wps.tile([128, 512], fp32, tag="psD", bufs=2)
                nc.tensor.matmul(psT, aTh_sb[:, ksl], bTh_sb[:, osl],
                                 start=True, stop=False)
                nc.tensor.matmul(psT, aTh_sb[:, ksl], bTl_sb[:, osl],
                                 start=False, stop=False)
                nc.tensor.matmul(psT, aTl_sb[:, ksl], bTh_sb[:, osl],
                                 start=False, stop=True)
                nc.tensor.matmul(psD, aD_sb[:, ksl], bD_sb[:, osl],
                                 start=True, stop=True)
                u = tmp.tile([128, 512], fp32, tag="u", bufs=2)
                sI = tmp.tile([128, 512], fp32, tag="sI", bufs=2)
                # t-clamp doubles as the PSUM->SBUF move; u is clamped
                # in-place on ScalarE so each engine carries 3 ops/tile
                nc.vector.tensor_scalar_max(sI, psT, 0.0)
                nc.vector.tensor_add(u, sI, psD)  # SBUF + PSUM (one-PSUM ok)
                nc.scalar.activation(u, u, af.Relu)
                nc.scalar.sqrt(sI, sI)
                nc.scalar.sqrt(u, u)
                nc.vector.tensor_sub(whs[oh][:, kt, :], u, sI)

    # --- main matmul, oh-phased: out[b, osl] = x[b, :] @ w[:, osl] ---
    if "nomm" in variant:
        return
    for oh in range(O_HALVES):
        osl = slice(oh * 512, (oh + 1) * 512)
        for bt in range(B_TILES):
            xt = xpool.tile([128, NUM_IN], fp16, tag="xt", bufs=8)
            nc.sync.dma_start(out=xt, in_=xT[bt])
            ps = mps.tile([128, 512], fp32, tag="ps", bufs=3)
            for kt in range(K_TILES):
                nc.tensor.matmul(
                    ps,
                    xt[:, kt * 128 : (kt + 1) * 128],
                    whs[oh][:, kt, :],
                    start=(kt == 0),
                    stop=(kt == K_TILES - 1),
                )
            ot = opool.tile([128, 512], fp32, tag="ot", bufs=3)
            nc.scalar.copy(ot, ps)
            nc.sync.dma_start(out=out[bt * 128 : (bt + 1) * 128, osl], in_=ot)


def _prep_inputs(inputs, init_in_pos, init_out_pos, in_pos, out_pos, biases):
    x = np.asarray(inputs, dtype=np.float32)
    a = np.asarray(in_pos, dtype=np.float64).reshape(NUM_IN, SD)
    a0 = np.asarray(init_in_pos, dtype=np.float64).reshape(NUM_IN, SD)
    b = np.asarray(out_pos, dtype=np.float64).reshape(NUM_OUT, SD)
    b0 = np.asarray(init_out_pos, dtype=np.float64).reshape(NUM_OUT, SD)
    bias = np.asarray(biases, dtype=np.float32).reshape(NUM_OUT)

    # xT[bt, d, kt*128+b'] = x[bt*128+b', kt*128+d]
    xT = np.ascontiguousarray(
        x.reshape(B_TILES, 128, K_TILES, 128).transpose(0, 3, 2, 1)
        .astype(np.float16)
    ).reshape(B_TILES, 128, NUM_IN)

    da, db = a - a0, b - b0
    Sa = (a * a).sum(1) - (a0 * a0).sum(1)
    Sb = (b * b).sum(1) - (b0 * b0).sum(1)

    ones_i = np.ones(NUM_IN)
    ones_o = np.ones(NUM_OUT)
    # D[i,o] = a.(-2db) + da.(-2b0) + Sa*1 + 1*Sb  = dist^2 - dist0^2
    aD = np.concatenate([a.T, da.T, Sa[None, :], ones_i[None, :]], 0)
    bD_full = np.concatenate([-2.0 * db.T, -2.0 * b0.T, ones_o[None, :],
                              Sb[None, :]], 0)
    # t[i,o] = a0.(-2b0) + |a0|^2*1 + 1*(|b0|^2+EPS) = dist0^2 + EPS,
    # via hi/lo split fp32r matmuls: t = ah.bh + ah.bl + al.bh
    aT = np.concatenate([a0.T, (a0 * a0).sum(1)[None, :], ones_i[None, :]], 0)
    bT_full = np.concatenate([-2.0 * b0.T, ones_o[None, :],
                              ((b0 * b0).sum(1) + EPS)[None, :]], 0)
    aTh, aTl = _split16(aT)
    bTh_full, bTl_full = _split16(bT_full)
    # pad 7-row pieces to 12 rows so everything packs into one [12,*] DMA
    pad_i = np.zeros((5, NUM_IN), np.float16)
    pad_o = np.zeros((5, NUM_OUT), np.float16)
    aTh12 = np.concatenate([aTh, pad_i], 0)
    aTl12 = np.concatenate([aTl, pad_i], 0)
    bTh12_full = np.concatenate([bTh_full, pad_o], 0)
    bTl12_full = np.concatenate([bTl_full, pad_o], 0)

    in_maps = []
    for c in range(N_CORES):
        sl = slice(c * O_SHARD, (c + 1) * O_SHARD)
        ab = np.ascontiguousarray(
            np.concatenate(
                [aD.astype(np.float16), aTh12, aTl12,
                 bD_full[:, sl].astype(np.float16),
                 bTh12_full[:, sl], bTl12_full[:, sl]], axis=1
            ).astype(np.float16)
        )
        in_maps.append({"xT": xT, "ab": ab})
    return in_maps, bias


def _run(in_maps, trace=False):
    from concourse.bass_utils import run_bass_kernel_spmd

    variant = os.environ.get("MESHFC_VARIANT", "")
    key = ("nc", variant)
    if key not in _CACHE:
        _CACHE[key] = _build_bass(variant)
    nc = _CACHE[key]
    res = run_bass_kernel_spmd(
        nc, in_maps, core_ids=list(range(N_CORES)), trace=trace
    )
    outs = [r["out"] for r in res.results]
    return np.concatenate(outs, axis=1), res


def kernel(**inputs) -> np.ndarray:
    in_maps, bias = _prep_inputs(**inputs)
    out, _ = _run(in_maps, trace=bool(os.environ.get("MESHFC_TRACE")))
    if bias.any():
        out = out + bias[None, :]
    return out


# revision 14
# speedup vs baseline: 1.4644x; 1.1457x over previous
"""MeshFC kernel for 8x TRN2 NeuronCores.

Computes: out = inputs @ w + biases, where
  w[i,o] = ||in_pos[i]-out_pos[o]|| - ||init_in_pos[i]-init_out_pos[o]||

Sharding: tensor-parallel on the output dim (8 x 1024 columns).

Per-core pipeline (all PE matmuls at 1 cycle/row):
  - weight gen via the difference form
        t = ||a0-b0||^2          u = t + D,  D = ||a-b||^2 - ||a0-b0||^2
        w = sqrt(u) - sqrt(t)
    t is computed from an fp16 hi/lo mantissa split (ah.bh + ah.bl + al.bh
    ~ 22-bit accuracy, err ~1e-5; single fp16/fp32r matmuls err at ~1e-3,
    swamping near-coincident pairs). D uses cancellation-free aug vectors
    (a, a-a0 paired with -2(b-b0), -2b0: every product is O(delta), so
    fp16 rounding costs only ~1e-5 on w). The three t terms (and for u
    also D) are stacked along the contraction dim into ONE K=128 matmul
    each, sharing one moving operand; zero rows blank out D for the t
    matmul. K is padded to 128 because the PE streams 1 cycle/row only
    for stationary K >= ~96 (2 cycles/row below - measured).
        w = sqrt(t+D) - sqrt(t)
    Both sqrt args are clamped to >=0 (the ~2e-5 residual t error can go
    negative on near-coincident pairs -> NaN otherwise; an eps shift is
    NOT usable: any eps creates sqrt(eps)-scale w errors on pairs with
    dist0^2 < eps). The t-error otherwise cancels between the two sqrts
    since u adds D on top of the same computed t.
  - main [4096,2048]x[2048,1024] matmul in fp16, oh-phased (512-col
    halves) so weight-gen clamp/sqrt/sub work (spread over DVE, GpSimd
    and ScalarE) overlaps the matmul stream; x streams through an 8-deep
    SBUF pool.
  - biases are added on the host (they are zeros in this problem).
"""

import os
from contextlib import ExitStack

import numpy as np

NUM_IN, NUM_OUT, SD, BATCH = 2048, 8192, 5, 4096
N_CORES = 8
O_SHARD = NUM_OUT // N_CORES  # 1024
B_TILES = BATCH // 128  # 32
K_TILES = NUM_IN // 128  # 16
O_HALVES = O_SHARD // 512  # 2
_CACHE = {}


def _split16(v):
    """hi/lo split at fp16 precision: v ~= hi + lo with both fp16."""
    hi = v.astype(np.float16)
    lo = (v - hi.astype(np.float64)).astype(np.float16)
    return hi, lo


def _build_bass(variant=""):
    import concourse.bass as bass  # noqa: F401
    import concourse.mybir as mybir
    from concourse import bacc
    from concourse.tile import TileContext

    fp32 = mybir.dt.float32
    fp32r = mybir.dt.float32r
    fp16 = mybir.dt.float16
    af = mybir.ActivationFunctionType

    nc = bacc.Bacc("TRN2", name="meshfc")

    xT = nc.dram_tensor("xT", [B_TILES, 128, NUM_IN], fp16, kind="ExternalInput")
    aU = nc.dram_tensor("aU", [128, NUM_IN], fp16, kind="ExternalInput")
    aT = nc.dram_tensor("aT", [128, NUM_IN], fp16, kind="ExternalInput")
    bU = nc.dram_tensor("bU", [128, O_SHARD], fp16, kind="ExternalInput")
    out = nc.dram_tensor("out", [BATCH, O_SHARD], fp32, kind="ExternalOutput")

    n_rep = 1
    for tok in variant.split(","):
        if tok.startswith("rep"):
            n_rep = int(tok[3:])

    with ExitStack() as ctx:
        tc = ctx.enter_context(TileContext(nc))
        const = ctx.enter_context(tc.tile_pool(name="const", bufs=1))
        wps = tmp = mps = opool = None
        if "nowgen" not in variant:
            wps = ctx.enter_context(tc.tile_pool(name="wps", bufs=2, space="PSUM"))
            tmp = ctx.enter_context(tc.tile_pool(name="tmp", bufs=2))
        if "nomm" not in variant:
            mps = ctx.enter_context(tc.tile_pool(name="mps", bufs=3, space="PSUM"))
            opool = ctx.enter_context(tc.tile_pool(name="op", bufs=3))

        # --- constants (K=128-stacked aug operands) ---
        aU_sb = const.tile([128, NUM_IN], fp16, name="aU_sb")
        aT_sb = const.tile([128, NUM_IN], fp16, name="aT_sb")
        bU_sb = const.tile([128, O_SHARD], fp16, name="bU_sb")
        nc.sync.dma_start(out=bU_sb, in_=bU[:, :])
        nc.sync.dma_start(out=aU_sb, in_=aU[:, :])
        nc.sync.dma_start(out=aT_sb, in_=aT[:, :])

        # x streamed through an 8-deep pool (32 KiB/part): each oh-phase
        # re-DMAs the 32 x tiles; prefetch stays ~8 tiles ahead of the PE
        xpool = None
        if "nomm" not in variant:
            xpool = ctx.enter_context(tc.tile_pool(name="xp", bufs=8))

        # per-half weight blocks (separate tiles so tile-level deps give
        # oh-granular pipelining even if subtile tracking is conservative)
        whs = [
            const.tile([128, K_TILES, 512], fp16, name=f"w{oh}")
            for oh in range(O_HALVES)
        ]

        for _rep in range(n_rep):
            _build_body(nc, variant, af, wps, tmp, mps, opool, xpool,
                        aU_sb, aT_sb, bU_sb, whs, xT, out, fp32, fp16)

    nc.finalize()
    return nc


def _build_body(nc, variant, af, wps, tmp, mps, opool, xpool,
                aU_sb, aT_sb, bU_sb, whs, xT, out, fp32, fp16):
    # --- weight generation: w = sqrt(u) - sqrt(t) ---
    if "nowgen" not in variant:
        for oh in range(O_HALVES):
            osl = slice(oh * 512, (oh + 1) * 512)
            for kt in range(K_TILES):
                ksl = slice(kt * 128, (kt + 1) * 128)
                psT = wps.tile([128, 512], fp32, tag="psT", bufs=2)
                psU = wps.tile([128, 512], fp32, tag="psU", bufs=2)
                nc.tensor.matmul(psT, aT_sb[:, ksl], bU_sb[:, osl],
                                 start=True, stop=True)
                nc.tensor.matmul(psU, aU_sb[:, ksl], bU_sb[:, osl],
                                 start=True, stop=True)
                u = tmp.tile([128, 512], fp32, tag="u", bufs=2)
                sI = tmp.tile([128, 512], fp32, tag="sI", bufs=2)
                # clamps to >=0 double as the PSUM->SBUF moves (GPSIMD
                # cannot read PSUM, so both run on DVE); ScalarE does the
                # two sqrts and GpSimd the SBUF-only sub: ~1.3us/tile chain
                # cadence vs the PE's 432ns/tile
                nc.vector.tensor_scalar_max(sI, psT, 0.0)
                nc.vector.tensor_scalar_max(u, psU, 0.0)
                nc.scalar.sqrt(sI, sI)
                nc.scalar.sqrt(u, u)
                nc.gpsimd.tensor_sub(whs[oh][:, kt, :], u, sI)

    # --- main matmul, oh-phased: out[b, osl] = x[b, :] @ w[:, osl] ---
    if "nomm" in variant:
        return
    for oh in range(O_HALVES):
        osl = slice(oh * 512, (oh + 1) * 512)
        for bt in range(B_TILES):
            xt = xpool.tile([128, NUM_IN], fp16, tag="xt", bufs=8)
            nc.sync.dma_start(out=xt, in_=xT[bt])
            ps = mps.tile([128, 512], fp32, tag="ps", bufs=3)
            for kt in range(K_TILES):
                nc.tensor.matmul(
                    ps,
                    xt[:, kt * 128 : (kt + 1) * 128],
                    whs[oh][:, kt, :],
                    start=(kt == 0),
                    stop=(kt == K_TILES - 1),
                )
            ot = opool.tile([128, 512], fp32, tag="ot", bufs=3)
            nc.scalar.copy(ot, ps)
            nc.sync.dma_start(out=out[bt * 128 : (bt + 1) * 128, osl], in_=ot)


def _prep_inputs(inputs, init_in_pos, init_out_pos, in_pos, out_pos, biases):
    x = np.asarray(inputs, dtype=np.float32)
    a = np.asarray(in_pos, dtype=np.float64).reshape(NUM_IN, SD)
    a0 = np.asarray(init_in_pos, dtype=np.float64).reshape(NUM_IN, SD)
    b = np.asarray(out_pos, dtype=np.float64).reshape(NUM_OUT, SD)
    b0 = np.asarray(init_out_pos, dtype=np.float64).reshape(NUM_OUT, SD)
    bias = np.asarray(biases, dtype=np.float32).reshape(NUM_OUT)

    # xT[bt, d, kt*128+b'] = x[bt*128+b', kt*128+d]
    xT = np.ascontiguousarray(
        x.reshape(B_TILES, 128, K_TILES, 128).transpose(0, 3, 2, 1)
        .astype(np.float16)
    ).reshape(B_TILES, 128, NUM_IN)

    da, db = a - a0, b - b0
    Sa = (a * a).sum(1) - (a0 * a0).sum(1)
    Sb = (b * b).sum(1) - (b0 * b0).sum(1)

    ones_i = np.ones(NUM_IN)
    ones_o = np.ones(NUM_OUT)
    # D[i,o] = a.(-2db) + da.(-2b0) + Sa*1 + 1*Sb  = dist^2 - dist0^2
    aD = np.concatenate([a.T, da.T, Sa[None, :], ones_i[None, :]], 0)
    bD_full = np.concatenate([-2.0 * db.T, -2.0 * b0.T, ones_o[None, :],
                              Sb[None, :]], 0)
    # t[i,o] = a0.(-2b0) + |a0|^2*1 + 1*|b0|^2 = dist0^2,
    # via fp16 hi/lo split: t = ah.bh + ah.bl + al.bh
    aT7 = np.concatenate([a0.T, (a0 * a0).sum(1)[None, :], ones_i[None, :]], 0)
    bT7_full = np.concatenate([-2.0 * b0.T, ones_o[None, :],
                               (b0 * b0).sum(1)[None, :]], 0)
    ah, al = _split16(aT7)
    bh_full, bl_full = _split16(bT7_full)
    # K=128 stacks sharing one moving operand bU = [bh|bl|bh|bD|0]:
    #   u-matmul stationary [ah|ah|al|aD|0] -> t + D
    #   t-matmul stationary [ah|ah|al| 0|0] -> t
    zpad = np.zeros((128 - 33, NUM_IN), np.float16)
    z12 = np.zeros((12, NUM_IN), np.float16)
    aUs = np.concatenate([ah, ah, al, aD.astype(np.float16), zpad], 0)
    aTs = np.concatenate([ah, ah, al, z12, zpad], 0)
    zpad_o = np.zeros((128 - 33, NUM_OUT), np.float16)
    bUs_full = np.concatenate([bh_full, bl_full, bh_full,
                               bD_full.astype(np.float16), zpad_o], 0)

    in_maps = []
    for c in range(N_CORES):
        sl = slice(c * O_SHARD, (c + 1) * O_SHARD)
        in_maps.append({
            "xT": xT,
            "aU": np.ascontiguousarray(aUs),
            "aT": np.ascontiguousarray(aTs),
            "bU": np.ascontiguousarray(bUs_full[:, sl]),
        })
    return in_maps, bias


def _run(in_maps, trace=False):
    from concourse.bass_utils import run_bass_kernel_spmd

    variant = os.environ.get("MESHFC_VARIANT", "")
    key = ("nc", variant)
    if key not in _CACHE:
        _CACHE[key] = _build_bass(variant)
    nc = _CACHE[key]
    res = run_bass_kernel_spmd(
        nc, in_maps, core_ids=list(range(N_CORES)), trace=trace
    )
    outs = [r["out"] for r in res.results]
    return np.concatenate(outs, axis=1), res


def kernel(**inputs) -> np.ndarray:
    in_maps, bias = _prep_inputs(**inputs)
    out, _ = _run(in_maps, trace=bool(os.environ.get("MESHFC_TRACE")))
    if bias.any():
        out = out + bias[None, :]
    return out
